# revision 21
# baseline (speedup 1.0000x reference)
"""Distributed multi-head self-attention for Trainium2 (8 NeuronCores).

Problem: b=4, n=2048, dim=1024, heads=16, dim_head=64.
  q = x@Wq; k,v = split(x@Wkv, 2); out = softmax(q k^T / 8) v; y = out@Wout + bout

Sharding: core c <-> (batch b=c//2, head-group g=c%2). Each core computes
q/k/v + attention for its batch's 8 heads (tensor-parallel columns of
Wq/Wkv). Pair (b,0)/(b,1) AllGathers transposed bf16 attention outputs
(full per pair for pairs 0-2, fired right after each pair finishes and
absorbed during the next pair; per-iq chunks for pair 3), mask-selects
its own half-sequence columns, then runs the output projection with the
full Wout over ITS HALF of the sequence. Core 2b+g emits out rows
[1024g : 1024(g+1)) of batch b.

The scalar engine (exp over 33.5M scores/core, ~1.15-1.33ns/col when fed
continuously) is the roofline; everything else is scheduled around it:
 - x and all weights are cast to bf16 on the HOST: weights DMA straight
   into their SBUF tiles and x^T is produced by 16 chunked XBAR DMA
   transposes (dma_start_transpose, 16x128 tiles) directly from DRAM -
   no staging, no PE transposes, no cast traffic on DVE/PSUM.
 - attention(0)'s first scores are emitted right after qkproj chunk 0
   (first exp ~15us); remaining qkproj(0) chunks + v-projections
   interleave with iq=0 steps in blocks, v-matmul flushes deferred
   within the at-ring depth so exp is never head-of-line blocked.
 - qkproj(p+1) drips 2-3 matmuls/step into attention(p) instead of
   stalling ACT with a 13us PE blob between pairs.
 - gather-selects drip two windows late (pair 0/1 in attention(2)) so a
   skewed AllGather can never head-of-line-block the DVE queue.
 - attention(3) runs iq order [0,2,1,3] with two-pass masked selects so
   the local block gated by chunks {0,2} projects mid-window; only the
   {1,3} block (4 m-tiles) + final AG chunk sit on the tail.
TensorEngine math is bf16 with f32 PSUM; scores run two heads
concurrently via tile_position row groups; softmax skips max-subtraction
(scores ~N(0,1)); denominators ride as a 65th ones-row of v; lazy
normalization off the critical path (reciprocal + gpsimd broadcast).
PSUM: psS 2x[128,1024]f32 (4 banks) + psO 2x[65,512] (2) + psP 2 = 8.
"""

import ml_dtypes
import numpy as np

import concourse.mybir as mybir
import concourse.tile as tile
from concourse import bacc, bass_utils

N_CORES = 8
B, N, D = 4, 2048, 1024
GH = 8          # heads per core
DH = 64
IN = GH * DH    # 512 inner dims per core
SCALE = DH ** -0.5
PT = 128
KD = D // PT    # 8 dim tiles
MS = N // PT    # 16 seq tiles
MI = IN // PT   # 4 head-pair tiles per core
NH = N // 2     # out rows per core
F32 = mybir.dt.float32
BF16 = mybir.dt.bfloat16
RG = [[0, 1], [2, 3], [4, 5], [6, 7]]

_COMPILED = None


def build():
    nc = bacc.Bacc("TRN2", target_bir_lowering=False, debug=False, num_devices=N_CORES)

    x_ext = nc.dram_tensor("x", [N, D], BF16, kind="ExternalInput")
    wq_ext = nc.dram_tensor("wq", [D, IN], BF16, kind="ExternalInput")
    wk_ext = nc.dram_tensor("wk", [D, IN], BF16, kind="ExternalInput")
    wv_ext = nc.dram_tensor("wv", [D, IN], BF16, kind="ExternalInput")
    wout_ext = nc.dram_tensor("wout", [D, D], BF16, kind="ExternalInput")
    bout_ext = nc.dram_tensor("bout", [D], F32, kind="ExternalInput")
    sel_ext = nc.dram_tensor("sel", [1, 2], F32, kind="ExternalInput")
    out_ext = nc.dram_tensor("out", [NH, D], F32, kind="ExternalOutput")

    with tile.TileContext(nc) as tc:
        with (
            tc.tile_pool(name="const", bufs=1) as constp,
            tc.tile_pool(name="wpool", bufs=1) as wpool,
            tc.tile_pool(name="qkv", bufs=1) as qkv,
            tc.tile_pool(name="attout", bufs=1) as attoutp,
            tc.tile_pool(name="xT", bufs=1) as xTp,
            tc.tile_pool(name="dram", bufs=1, space="DRAM") as dram,
        ):
            bias_row = constp.tile([1, D], F32)
            nc.sync.dma_start(bias_row[:], bout_ext[None, :])
            bias_bf = constp.tile([1, D], BF16)
            nc.vector.tensor_copy(bias_bf[:], bias_row[:])
            ones_col = constp.tile([1, PT], BF16)
            nc.gpsimd.memset(ones_col[:], 1.0)
            sel_row = constp.tile([1, 2], F32)
            nc.sync.dma_start(sel_row[:], sel_ext[:])
            s0_bc = constp.tile([PT, 1], F32)
            s1_bc = constp.tile([PT, 1], F32)
            nc.gpsimd.partition_broadcast(s0_bc[:], sel_row[:, 0:1])
            nc.gpsimd.partition_broadcast(s1_bc[:], sel_row[:, 1:2])

            wq_all = wpool.tile([PT, KD, IN], BF16, name="wq_all")
            wk_all = wpool.tile([PT, KD, IN], BF16, name="wk_all")
            wv_all = wpool.tile([PT, KD, IN], BF16, name="wv_all")
            wo_all = wpool.tile([PT, KD, D], BF16, name="wo_all")

            xTall = xTp.tile([PT, KD, N], BF16, name="xTall")
            qT = [qkv.tile([PT, N], BF16, name=f"qT{m}") for m in range(MI)]
            kT = [qkv.tile([PT, N], BF16, name=f"kT{m}") for m in range(MI)]
            vsb = [qkv.tile([PT, GH, 66], BF16, name=f"v{s}") for s in range(MS)]
            attoutT = [attoutp.tile([PT, N], BF16, name=f"attoutT{p}") for p in range(MI)]
            # after AG(p) the attoutT[p] data is snapshotted to DRAM; its
            # halves are reused for the mask-selected gathered k-tiles
            attThalf = [
                attoutT[k % MI][:, (k // MI) * NH:(k // MI + 1) * NH]
                for k in range(KD)
            ]

            ag_in = [dram.tile([PT, N], BF16, name=f"ag_in{p}") for p in range(3)]
            ag_out = [dram.tile([2 * PT, N], BF16, name=f"ag_out{p}") for p in range(3)]
            ag_cin = [dram.tile([PT, 512], BF16, name=f"ag_cin{i}") for i in range(4)]
            ag_chunk = [dram.tile([2 * PT, 512], BF16, name=f"ag_chunk{i}")
                        for i in range(4)]

            with (
                tc.tile_pool(name="psS", bufs=2, space="PSUM") as psS,
                tc.tile_pool(name="psO", bufs=2, space="PSUM") as psO,
                tc.tile_pool(name="psP", bufs=2, space="PSUM") as psP,
                tc.tile_pool(name="attn", bufs=8) as attnp,
                tc.tile_pool(name="fin", bufs=2) as finp,
                tc.tile_pool(name="osb", bufs=3) as osbp,
                tc.tile_pool(name="agst", bufs=2) as agst,
            ):
                # ---------- emission helpers ----------
                qkproj_ph = [None]

                def qkproj_mm(p, ch, i):
                    """i-th item (0..17) of pair p's chunk ch: 8 q-mms,
                    q-copy, 8 k-mms, k-copy"""
                    if i == 8 or i == 17:
                        dstT = qT if i == 8 else kT
                        nc.vector.tensor_copy(
                            dstT[p][:, ch * 512:(ch + 1) * 512], qkproj_ph[0][:]
                        )
                        return
                    k = i if i < 8 else i - 9
                    w_all = wq_all if i < 8 else wk_all
                    if k == 0:
                        qkproj_ph[0] = psP.tile([PT, 512], F32, name="ph", tag="psP")
                    nc.tensor.matmul(
                        qkproj_ph[0][:],
                        w_all[:, k, p * PT:(p + 1) * PT],
                        xTall[:, k, ch * 512:(ch + 1) * 512],
                        start=(k == 0), stop=(k == KD - 1),
                    )

                def qkproj_chunk(p, ch):
                    for i in range(18):
                        qkproj_mm(p, ch, i)

                def vproj_s(s):
                    pv = psP.tile([PT, 512], F32, name="pv", tag="psP")
                    for k in range(KD):
                        nc.tensor.matmul(
                            pv[:],
                            xTall[:, k, s * PT:(s + 1) * PT],
                            wv_all[:, k, :],
                            start=(k == 0), stop=(k == KD - 1),
                        )
                    nc.gpsimd.memset(vsb[s][:, :, 64:65], 1.0)
                    nc.vector.tensor_copy(
                        vsb[s][:, :, 0:64],
                        pv[:].rearrange("p (h e) -> p h e", h=GH),
                    )

                # ---------- gathers + selects ----------
                def ag_full(p):
                    nc.sync.dma_start(ag_in[p][:], attoutT[p][:])
                    nc.gpsimd.collective_compute(
                        "AllGather", mybir.AluOpType.bypass,
                        replica_groups=RG,
                        ins=[ag_in[p].opt()], outs=[ag_out[p].opt()],
                    )

                def select_items(p):
                    """mask-select pair p's gathered halves into attThalf"""
                    items = []
                    for kk in (p, p + MI):
                        def sel(p=p, kk=kk):
                            half = kk // MI
                            ast = agst.tile([PT, N], BF16, name="ast", tag="ast")
                            nc.sync.dma_start(
                                ast[:], ag_out[p][half * PT:(half + 1) * PT, :]
                            )
                            tmp = agst.tile([PT, NH], BF16, name="tmp", tag="tmp")
                            nc.vector.tensor_scalar_mul(
                                tmp[:], ast[:, 0:NH], s0_bc[:]
                            )
                            nc.vector.scalar_tensor_tensor(
                                attThalf[kk],
                                ast[:, NH:N], s1_bc[:], tmp[:],
                                op0=mybir.AluOpType.mult,
                                op1=mybir.AluOpType.add,
                            )

                        items.append(sel)
                    return items

                def ag_iq(iq):
                    nc.sync.dma_start(
                        ag_cin[iq][:], attoutT[3][:, iq * 512:(iq + 1) * 512]
                    )
                    nc.gpsimd.collective_compute(
                        "AllGather", mybir.AluOpType.bypass,
                        replica_groups=RG,
                        ins=[ag_cin[iq].opt()], outs=[ag_chunk[iq].opt()],
                    )

                def select3_pass(b, second):
                    """block b of pair 3's select: pass1 = s0*chunk[b],
                    pass2 adds s1*chunk[2+b]"""
                    items = []
                    for kk in (3, 7):
                        def sel(b=b, kk=kk, second=second):
                            half = kk // MI
                            src = ag_chunk[2 + b if second else b]
                            astc = agst.tile([PT, 512], BF16, name="astc",
                                             tag="astc")
                            nc.sync.dma_start(
                                astc[:], src[half * PT:(half + 1) * PT, :]
                            )
                            dst = attThalf[kk][:, b * 512:(b + 1) * 512]
                            if not second:
                                nc.vector.tensor_scalar_mul(
                                    dst, astc[:], s0_bc[:]
                                )
                            else:
                                nc.vector.scalar_tensor_tensor(
                                    dst, astc[:], s1_bc[:], dst,
                                    op0=mybir.AluOpType.mult,
                                    op1=mybir.AluOpType.add,
                                )

                        items.append(sel)
                    return items

                # ---------- output projection over own half ----------
                KORDER = [0, 4, 1, 5, 2, 6, 3, 7]
                proj_ps = [None]

                def proj_items(ms, use_scalar=False):
                    items = []
                    for m in ms:
                        for nn in range(2):
                            def grp(m=m, nn=nn):
                                ps = psP.tile([PT, 512], F32, name="po", tag="psP")
                                proj_ps[0] = ps
                                nc.tensor.matmul(
                                    ps[:], ones_col[:],
                                    bias_bf[:, nn * 512:(nn + 1) * 512],
                                    start=True, stop=False,
                                )
                                for ki, kk in enumerate(KORDER):
                                    nc.tensor.matmul(
                                        ps[:],
                                        attThalf[kk][:, m * PT:(m + 1) * PT],
                                        wo_all[:, kk, nn * 512:(nn + 1) * 512],
                                        start=False, stop=(ki == KD - 1),
                                    )

                            def cpy(m=m, nn=nn, use_scalar=use_scalar):
                                osb = osbp.tile([PT, 512], F32, name="osb",
                                                tag="osb")
                                if use_scalar:
                                    nc.scalar.copy(osb[:], proj_ps[0][:])
                                else:
                                    nc.vector.tensor_copy(osb[:], proj_ps[0][:])
                                nc.sync.dma_start(
                                    out_ext[m * PT:(m + 1) * PT,
                                            nn * 512:(nn + 1) * 512],
                                    osb[:],
                                )

                            items.append(grp)
                            items.append(cpy)
                    return items

                # ---------- attention ----------
                def att_step(p, iq, j, state):
                    ps = psS.tile([PT, 1024], F32, name="ps", tag="psS")
                    nc.tensor.matmul(
                        ps[:, 0:512],
                        kT[p][0:64, j * PT:(j + 1) * PT],
                        qT[p][0:64, iq * 512:(iq + 1) * 512],
                        start=True, stop=True,
                        tile_position=(0, 0),
                    )
                    nc.tensor.matmul(
                        ps[:, 512:1024],
                        kT[p][64:128, j * PT:(j + 1) * PT],
                        qT[p][64:128, iq * 512:(iq + 1) * 512],
                        start=True, stop=True,
                        tile_position=(64, 0),
                    )
                    at = attnp.tile([PT, 1024], BF16, name="at", tag="at")
                    nc.scalar.activation(
                        at[:], ps[:], mybir.ActivationFunctionType.Exp,
                        scale=SCALE,
                    )
                    return (iq, j, at)

                def att_flush(p, state, on_finalize=None):
                    outs, pend = state["outs"], state["pend"]
                    if pend is not None:
                        emit_vmm(p, outs, *pend)
                        if pend[1] == MS - 1:
                            finalize(p, outs, pend[0])
                            if on_finalize is not None:
                                on_finalize(pend[0])
                        state["pend"] = None

                def attention(p, hook=None, on_finalize=None,
                              iq_order=(0, 1, 2, 3), state=None):
                    steps = [(iq, j) for iq in iq_order for j in range(MS)]
                    if state is None:
                        state = {"outs": {}, "pend": None}
                    else:
                        steps = [s_ for s_ in steps if s_[0] != 0]
                    for si, (iq, j) in enumerate(steps):
                        nxt = att_step(p, iq, j, state)
                        if hook is not None:
                            hook(si, iq, j)
                        att_flush(p, state, on_finalize)
                        state["pend"] = nxt
                    att_flush(p, state, on_finalize)

                def emit_vmm(p, outs, iq, j, at):
                    if j == 0:
                        outs[iq] = (
                            psO.tile([65, 512], F32, name="oA", tag="psO"),
                            psO.tile([65, 512], F32, name="oB", tag="psO"),
                        )
                    oA, oB = outs[iq]
                    nc.tensor.matmul(
                        oA[:], vsb[j][:, 2 * p, 0:65], at[:, 0:512],
                        start=(j == 0), stop=(j == MS - 1),
                    )
                    nc.tensor.matmul(
                        oB[:], vsb[j][:, 2 * p + 1, 0:65], at[:, 512:1024],
                        start=(j == 0), stop=(j == MS - 1),
                    )

                def finalize(p, outs, iq):
                    dens = []
                    for hh, o in enumerate(outs[iq]):
                        seg = attoutT[p][hh * 64:(hh + 1) * 64,
                                         iq * 512:(iq + 1) * 512]
                        nc.vector.tensor_copy(seg, o[0:64, :])
                        den = finp.tile([1, 512], F32, name="den", tag="den")
                        nc.vector.tensor_copy(den[:], o[64:65, :])
                        dens.append((hh, den))
                    for hh, den in dens:
                        recip = finp.tile([1, 512], F32, name="recip", tag="recip")
                        nc.vector.reciprocal_approx_fast(recip[:], den[:])
                        bc = finp.tile([PT, 512], F32, name="bc", tag="bc")
                        nc.gpsimd.partition_broadcast(bc[:], recip[:])
                        seg = attoutT[p][hh * 64:(hh + 1) * 64,
                                         iq * 512:(iq + 1) * 512]
                        nc.vector.tensor_tensor(
                            seg, seg, bc[hh * 64:(hh + 1) * 64, :],
                            op=mybir.AluOpType.mult,
                        )

                # ---------- drip machinery ----------
                drip = []

                def drip_pump(k):
                    for _ in range(min(k, len(drip))):
                        drip.pop(0)()

                def queue_qkproj(p):
                    for ch in range(4):
                        for i in range(18):
                            drip.append(lambda p=p, ch=ch, i=i: qkproj_mm(p, ch, i))

                # ---------- phase 0 head, interleaved with iq=0 steps ----------
                nc.sync.dma_start(
                    wq_all[:], wq_ext.rearrange("(a b) c -> b a c", b=PT)
                )
                nc.sync.dma_start(
                    wk_all[:], wk_ext.rearrange("(a b) c -> b a c", b=PT)
                )
                nc.sync.dma_start_transpose(
                    xTall[:, :, 0:512], x_ext[0:512, :]
                )

                st0 = {"outs": {}, "pend": None}
                qkproj_chunk(0, 0)
                nc.sync.dma_start(
                    wv_all[:], wv_ext.rearrange("(a b) c -> b a c", b=PT)
                )
                for sb in range(1, 4):
                    nc.sync.dma_start_transpose(
                        xTall[:, :, sb * 512:(sb + 1) * 512],
                        x_ext[sb * 512:(sb + 1) * 512, :],
                    )
                # scores j=0-3 need only x chunks 0-3 + qk chunk 0; their
                # v-matmul flushes defer until vproj lands (at ring = 8)
                pends = []
                for j in range(4):
                    pends.append(att_step(0, 0, j, st0))
                for blk in range(3):
                    qkproj_chunk(0, blk + 1)
                    for s in range(4 * blk, 4 * blk + 4):
                        vproj_s(s)
                    for pd in pends:
                        st0["pend"] = pd
                        att_flush(0, st0)
                    pends = []
                    for j in range(4 * blk + 4, 4 * blk + 8):
                        pends.append(att_step(0, 0, j, st0))
                for s in range(12, 16):
                    vproj_s(s)
                for pd in pends:
                    st0["pend"] = pd
                    att_flush(0, st0)

                # ---------- attention(0) iq 1-3: wout DMA + qkproj(1) ----------
                def hook0(si, iq, j):
                    if iq == 2 and j == 0:
                        nc.sync.dma_start(
                            wo_all[:], wout_ext.rearrange("(a b) c -> b a c", b=PT)
                        )
                    drip_pump(3)

                queue_qkproj(1)
                attention(0, hook=hook0, state=st0)
                drip_pump(len(drip))
                ag_full(0)

                queue_qkproj(2)
                attention(1, hook=lambda si, iq, j: drip_pump(2))
                drip_pump(len(drip))
                ag_full(1)

                queue_qkproj(3)
                drip.extend(select_items(0))
                drip.extend(select_items(1))
                attention(2, hook=lambda si, iq, j: drip_pump(2))
                drip_pump(len(drip))
                ag_full(2)

                # ------- attention(3): chunked AG + selects + proj -------
                def on_fin3(iq):
                    ag_iq(iq)
                    if iq == 2:
                        # both b0 chunks {0,2} gathered AND iq2 columns of
                        # attoutT[3] snapshotted: full b0 selects are safe
                        drip.extend(select3_pass(0, second=False))
                        drip.extend(select3_pass(0, second=True))
                        drip.extend(proj_items(range(0, 4)))

                drip.extend(select_items(2))
                attention(3, hook=lambda si, iq, j: drip_pump(2),
                          on_finalize=on_fin3, iq_order=(0, 2, 1, 3))
                drip_pump(len(drip))
                for it in select3_pass(1, second=False):
                    it()
                for it in select3_pass(1, second=True):
                    it()
                for it in proj_items(range(4, 8), use_scalar=True):
                    it()

    nc.compile()
    return nc


def _shard_inputs(x, Wq, Wkv, Wout, bout):
    bf = ml_dtypes.bfloat16
    in_maps = []
    for c in range(N_CORES):
        b, g = c // 2, c % 2
        sel = np.zeros((1, 2), dtype=np.float32)
        sel[0, g] = 1.0
        in_maps.append({
            "x": np.ascontiguousarray(x[b]).astype(bf),
            "wq": np.ascontiguousarray(Wq[:, g * IN:(g + 1) * IN]).astype(bf),
            "wk": np.ascontiguousarray(Wkv[:, g * IN:(g + 1) * IN]).astype(bf),
            "wv": np.ascontiguousarray(
                Wkv[:, D + g * IN:D + (g + 1) * IN]
            ).astype(bf),
            "wout": np.ascontiguousarray(Wout).astype(bf),
            "bout": np.ascontiguousarray(bout, dtype=np.float32),
            "sel": sel,
        })
    return in_maps


def kernel(x, Wq, Wkv, Wout, bout):
    global _COMPILED
    if _COMPILED is None:
        _COMPILED = build()
    nc = _COMPILED
    in_maps = _shard_inputs(
        np.asarray(x), np.asarray(Wq), np.asarray(Wkv), np.asarray(Wout),
        np.asarray(bout),
    )
    res = bass_utils.run_bass_kernel_spmd(nc, in_maps, core_ids=list(range(N_CORES)))
    out = np.empty((B, N, D), dtype=np.float32)
    for c in range(N_CORES):
        b, g = c // 2, c % 2
        out[b, g * NH:(g + 1) * NH, :] = res.results[c]["out"]
    return out


if __name__ == "__main__":
    rng = np.random.default_rng(0)
    x = rng.standard_normal((B, N, D)).astype(np.float32)
    Wq = rng.standard_normal((D, D)).astype(np.float32) * D ** -0.5
    Wkv = rng.standard_normal((D, 2 * D)).astype(np.float32) * D ** -0.5
    Wout = rng.standard_normal((D, D)).astype(np.float32) * D ** -0.5
    bout = np.zeros((D,), dtype=np.float32)
    y = kernel(x=x, Wq=Wq, Wkv=Wkv, Wout=Wout, bout=bout)
    print("out shape:", y.shape, "finite:", np.isfinite(y).all())
